# revision 8
# baseline (speedup 1.0000x reference)
"""Trainium2 Bass kernel for Gaussian KDE evaluation.

reference math:
    val[m] = (1/N) * sum_n exp(t1 - 0.5*d2(m,n)/bw^2)
    d2(m,n) = |e_m|^2 + |b_n|^2 - 2<e_m, b_n>
    t1 = -0.5*D*log(2*pi) - log_bw,  bw^2 = exp(2*log_bw)

Strategy (8 NeuronCores, x_eval row-sharded, x_base/log_bw replicated):
  All matmuls run in fp16 (1 PE cycle/row vs 4 for fp32) using an exact
  hi/lo error split so d2 keeps ~2^-22 effective precision:
    cross  = he*hb + le*hb + he*lb   (he=f16(-2e), le=f16(-2e-he), ...)
    |b|^2  = ones*P_hi + ones*P_lo   (P_hi=f16(b^2), P_lo=f16(b^2-P_hi))
  One K=80 matmul per 512-col tile produces |b|^2 - 2<e,b> in PSUM; a
  single ScalarE ACTIVATE computes exp(scale*psum + bias) in place with
  scale = -0.5/bw^2, per-partition bias = t1 - ln(N) + scale*|e_m|^2,
  and its accum_out emits the row-sum.  Base columns stream per chunk:
  DMA slice -> DVE casts -> PE transposes -> rhs tile, fully overlapped
  with the ACT-bound main loop.
"""

import numpy as np

M, N, D = 8192, 16384, 16
NCORES = 8
MS = M // NCORES          # eval rows per core
RT = MS // 128            # row tiles per core (128 evals each)
CH = 1536                 # column-chunk size (3 PSUM banks)
TPC = CH // D // 8        # nominal tiles per chunk / 8
LOG_2PI = float(np.log(2.0 * np.pi))

_CACHE = {}


def _chunks():
    out = []
    c0 = 0
    while c0 < N:
        csz = min(CH, N - c0)
        out.append((c0, csz))
        c0 += csz
    return out


def _build_nc(reps=1, loop_iters=None, skip_act=False, skip_mm=False,
              skip_tp=False, max_chunks=None):
    from concourse import bacc, mybir, masks, tile

    f32 = mybir.dt.float32
    f16 = mybir.dt.float16
    nc = bacc.Bacc("TRN2", target_bir_lowering=False, debug=False,
                   num_devices=NCORES)

    x_eval = nc.dram_tensor("x_eval", [MS, D], f32, kind="ExternalInput")
    x_base = nc.dram_tensor("x_base", [N, D], f32, kind="ExternalInput")
    log_bw = nc.dram_tensor("log_bw", [1, 1], f32, kind="ExternalInput")
    out = nc.dram_tensor("out", [128, RT], f32, kind="ExternalOutput")

    chunks = _chunks()
    NCH = len(chunks)
    Exp = mybir.ActivationFunctionType.Exp
    ADD = mybir.AluOpType.add
    MULT = mybir.AluOpType.mult
    X = mybir.AxisListType.X
    # constant part of the bias: t1 - ln(N), except -log_bw added on-device
    c0 = -0.5 * D * LOG_2PI - float(np.log(N))

    with tile.TileContext(nc) as tc:
        with (
            tc.tile_pool(name="persist", bufs=1) as pp,
            tc.tile_pool(name="bs", bufs=2) as bsp,
            tc.tile_pool(name="rhs", bufs=2) as rhsp,
            tc.tile_pool(name="mm", bufs=2, space="PSUM") as mmp,
            tc.tile_pool(name="tp", bufs=2, space="PSUM") as tpp,
        ):
          from contextlib import nullcontext
          for _rep in range(reps):
           with (tc.For_i(0, loop_iters, 1) if loop_iters else nullcontext()):
            identity = pp.tile([128, 128], f16)
            masks.make_identity(nc, identity[:])

            # ---- log_bw -> per-partition scale/bias columns -------------
            ones_row = pp.tile([1, 128], f32)
            nc.vector.memset(ones_row[:], 1.0)
            lb_sb = pp.tile([1, 1], f32)
            nc.sync.dma_start(out=lb_sb[:], in_=log_bw[:])
            ps_lb = mmp.tile([128, CH], f32, tag="mm")
            nc.tensor.matmul(ps_lb[:, 0:1], ones_row[:], lb_sb[:],
                             start=True, stop=True)
            lb_col = pp.tile([128, 1], f32)
            nc.vector.tensor_copy(lb_col[:], ps_lb[:, 0:1])
            # scale = -0.5 * exp(-2*log_bw)
            inv_bw2 = pp.tile([128, 1], f32)
            nc.scalar.activation(inv_bw2[:], lb_col[:], Exp, scale=-2.0)
            scale_col = pp.tile([128, 1], f32)
            nc.vector.tensor_scalar_mul(scale_col[:], inv_bw2[:], -0.5)
            # c_col = c0 - log_bw
            c_col = pp.tile([128, 1], f32)
            nc.vector.tensor_scalar(out=c_col[:], in0=lb_col[:],
                                    scalar1=-1.0, scalar2=c0,
                                    op0=MULT, op1=ADD)

            # ---- eval-side setup ----------------------------------------
            ev_nat = pp.tile([128, RT * D], f32)
            nc.sync.dma_start(
                out=ev_nat[:].rearrange("p (t d) -> p t d", d=D),
                in_=x_eval[:].rearrange("(p t) d -> p t d", p=128))
            ev_sq = pp.tile([128, RT * D], f32)
            nc.vector.tensor_mul(ev_sq[:], ev_nat[:], ev_nat[:])
            sq_e = pp.tile([128, RT], f32)
            nc.vector.tensor_reduce(
                out=sq_e[:], in_=ev_sq[:].rearrange("p (t d) -> p t d", d=D),
                axis=X, op=ADD)
            # bias_all[:, rt] = scale*|e|^2 + (c0 - log_bw)
            bias_all = pp.tile([128, RT], f32)
            nc.vector.tensor_scalar(out=bias_all[:], in0=sq_e[:],
                                    scalar1=scale_col[:, 0:1],
                                    scalar2=c_col[:, 0:1],
                                    op0=MULT, op1=ADD)

            # hi/lo split of -2*eval, packed for 32-row stacked transposes:
            #   ee_nat[p, rt, 0:16]=he, [16:32]=he   -> evT16 rows 0:32
            #   lo_nat[p, rt, 0:16]=le, [16:32]=1.0  -> evT16 rows 32:64
            e2f = pp.tile([128, RT * D], f32)
            nc.vector.tensor_scalar_mul(e2f[:], ev_nat[:], -2.0)
            ee_nat = pp.tile([128, RT * 2 * D], f16)
            ee_r = ee_nat[:].rearrange("p (t d) -> p t d", d=2 * D)
            e2f_r = e2f[:].rearrange("p (t d) -> p t d", d=D)
            nc.vector.tensor_copy(ee_r[:, :, 0:D], e2f_r)
            nc.vector.tensor_copy(ee_r[:, :, D:2 * D], ee_r[:, :, 0:D])
            e2hf = pp.tile([128, RT * D], f32)
            nc.vector.tensor_copy(e2hf[:], ee_r[:, :, 0:D])
            lo_nat = pp.tile([128, RT * 2 * D], f16)
            nc.vector.memset(lo_nat[:], 1.0)
            lo_r = lo_nat[:].rearrange("p (t d) -> p t d", d=2 * D)
            nc.vector.tensor_sub(lo_r[:, :, 0:D], e2f[:].rearrange(
                "p (t d) -> p t d", d=D), e2hf[:].rearrange(
                "p (t d) -> p t d", d=D))

            # evT16 rows: 0:16 he, 16:32 he, 32:48 le, 48:80 ones
            evT16 = pp.tile([80, MS], f16)
            nc.vector.memset(evT16[:], 1.0)
            tpe_h = tpp.tile([32, 1024], f16, tag="tp")
            tpe_l = tpp.tile([32, 1024], f16, tag="tp")
            for rt in range(RT):
                if not skip_tp:
                    nc.tensor.transpose(tpe_h[:, rt * 128:(rt + 1) * 128],
                                        ee_nat[:, rt * 32:(rt + 1) * 32],
                                        identity[:])
                    nc.tensor.transpose(tpe_l[:, rt * 128:(rt + 1) * 128],
                                        lo_nat[:, rt * 32:(rt + 1) * 32],
                                        identity[:])
            nc.vector.tensor_copy(evT16[0:32, :], tpe_h[:])
            nc.vector.tensor_copy(evT16[32:64, :], tpe_l[:])

            # ---- main loop: stream base columns per chunk ---------------
            sums = pp.tile([128, RT * NCH], f32)
            if skip_act or (max_chunks is not None and max_chunks < NCH):
                nc.vector.memset(sums[:], 0.0)
            xb_r = x_base[:].rearrange("(p t) d -> p t d", p=128)
            for ci, (cs, csz) in enumerate(chunks[:max_chunks]):
                nt = csz // 128
                t0 = cs // 128
                bs_c = bsp.tile([128, 12 * D], f32, tag="bs")
                nc.sync.dma_start(
                    out=bs_c[:, 0:nt * D].rearrange("p (t d) -> p t d", d=D),
                    in_=xb_r[:, t0:t0 + nt, :])
                w = nt * D
                bs_r = bs_c[:, 0:w].rearrange("p (t d) -> p t d", d=D)
                # hl[p, t, 0:16]=hb, [16:32]=lb  -> rhs rows 0:32
                # hp[p, t, 0:16]=hb, [16:32]=ph  -> rhs rows 32:64
                # pl[p, t, 0:16]=pl              -> rhs rows 64:80
                hl = bsp.tile([128, 12 * 2 * D], f16, tag="hl")
                hl_r = hl[:, 0:2 * w].rearrange("p (t d) -> p t d", d=2 * D)
                nc.vector.tensor_copy(hl_r[:, :, 0:D], bs_r)
                hbf = bsp.tile([128, 12 * D], f32, tag="hbf")
                hbf_r = hbf[:, 0:w].rearrange("p (t d) -> p t d", d=D)
                nc.vector.tensor_copy(hbf_r, hl_r[:, :, 0:D])
                nc.vector.tensor_sub(hl_r[:, :, D:2 * D], bs_r, hbf_r)
                hp = bsp.tile([128, 12 * 2 * D], f16, tag="hp")
                hp_r = hp[:, 0:2 * w].rearrange("p (t d) -> p t d", d=2 * D)
                nc.vector.tensor_copy(hp_r[:, :, 0:D], hl_r[:, :, 0:D])
                b2 = bsp.tile([128, 12 * D], f32, tag="b2")
                nc.vector.tensor_mul(b2[:, 0:w], bs_c[:, 0:w], bs_c[:, 0:w])
                b2_r = b2[:, 0:w].rearrange("p (t d) -> p t d", d=D)
                nc.vector.tensor_copy(hp_r[:, :, D:2 * D], b2_r)
                phf = bsp.tile([128, 12 * D], f32, tag="phf")
                phf_r = phf[:, 0:w].rearrange("p (t d) -> p t d", d=D)
                nc.vector.tensor_copy(phf_r, hp_r[:, :, D:2 * D])
                pl = bsp.tile([128, 12 * D], f16, tag="pl")
                nc.vector.tensor_sub(pl[:, 0:w], b2[:, 0:w], phf[:, 0:w])

                rhs = rhsp.tile([80, CH], f16, tag="rhs")
                for src, dst, sw in ((hl, 0, 2 * D), (hp, 32, 2 * D),
                                     (pl, 64, D)):
                    np_t = 16 * (sw // D)  # out partitions per transpose
                    for b0 in range(0, nt, 8):
                        bw_t = min(8, nt - b0)
                        tp = tpp.tile([32, 1024], f16, tag="tp")
                        for j in range(bw_t):
                            if not skip_tp:
                                nc.tensor.transpose(
                                    tp[0:np_t, j * 128:(j + 1) * 128],
                                    src[:, (b0 + j) * sw:(b0 + j + 1) * sw],
                                    identity[:])
                        nc.vector.tensor_copy(
                            rhs[dst:dst + np_t,
                                b0 * 128:(b0 + bw_t) * 128],
                            tp[0:np_t, 0:bw_t * 128])
                for rt in range(RT):
                    ps = mmp.tile([128, CH], f32, tag="mm")
                    if not skip_mm:
                        for j in range(csz // 512):
                            nc.tensor.matmul(
                                ps[:, j * 512:(j + 1) * 512],
                                evT16[0:80, rt * 128:(rt + 1) * 128],
                                rhs[0:80, j * 512:(j + 1) * 512],
                                start=True, stop=True)
                    if not skip_act:
                        nc.scalar.activation(
                            ps[:, 0:csz], ps[:, 0:csz], Exp,
                            bias=bias_all[:, rt:rt + 1],
                            scale=scale_col[:, 0:1],
                            accum_out=sums[:, rt * NCH + ci:rt * NCH + ci + 1])

            # ---- finalize -----------------------------------------------
            val = pp.tile([128, RT], f32)
            for rt in range(RT):
                nc.vector.tensor_reduce(
                    out=val[:, rt:rt + 1],
                    in_=sums[:, rt * NCH:(rt + 1) * NCH], axis=X, op=ADD)
            nc.sync.dma_start(out=out[:], in_=val[:])

    nc.compile()
    return nc


def kernel(x_eval, x_base, log_bw):
    from concourse.bass_utils import run_bass_kernel_spmd

    if "nc" not in _CACHE:
        _CACHE["nc"] = _build_nc()
    nc = _CACHE["nc"]

    x_eval = np.ascontiguousarray(x_eval, dtype=np.float32)
    x_base = np.ascontiguousarray(x_base, dtype=np.float32)
    lb = np.asarray(log_bw, dtype=np.float32).reshape(1, 1)
    in_maps = [
        {
            "x_eval": x_eval[i * MS:(i + 1) * MS],
            "x_base": x_base,
            "log_bw": lb,
        }
        for i in range(NCORES)
    ]
    res = run_bass_kernel_spmd(nc, in_maps, list(range(NCORES)))
    # out[p, rt] holds eval point p*RT + rt of the shard -> row-major flatten
    shards = [r["out"].reshape(-1) for r in res.results]
    return np.concatenate(shards).astype(np.float32)


# revision 9
# speedup vs baseline: 1.0935x; 1.0935x over previous
"""Trainium2 Bass kernel for Gaussian KDE evaluation.

reference math:
    val[m] = (1/N) * sum_n exp(t1 - 0.5*d2(m,n)/bw^2)
    d2(m,n) = |e_m|^2 + |b_n|^2 - 2<e_m, b_n>
    t1 = -0.5*D*log(2*pi) - log_bw,  bw^2 = exp(2*log_bw)

Strategy (8 NeuronCores, x_eval row-sharded, x_base/log_bw replicated):
  All matmuls run in fp16 (1 PE cycle/row vs 4 for fp32) using an exact
  hi/lo error split so d2 keeps ~2^-22 effective precision:
    cross  = he*hb + le*hb + he*lb   (he=f16(-2e), le=f16(-2e-he), ...)
    |b|^2  = ones*P_hi + ones*P_lo   (P_hi=f16(b^2), P_lo=f16(b^2-P_hi))
  One K=80 matmul per 512-col tile produces |b|^2 - 2<e,b> in PSUM; a
  single ScalarE ACTIVATE computes exp(scale*psum + bias) in place with
  scale = -0.5/bw^2, per-partition bias = t1 - ln(N) + scale*|e_m|^2,
  and its accum_out emits the row-sum.  Base columns stream per chunk:
  DMA slice -> DVE casts -> PE transposes -> rhs tile, fully overlapped
  with the ACT-bound main loop.
"""

import numpy as np

M, N, D = 8192, 16384, 16
NCORES = 8
MS = M // NCORES          # eval rows per core
RT = MS // 128            # row tiles per core (128 evals each)
CH = 1536                 # column-chunk size (3 PSUM banks)
TPC = CH // D // 8        # nominal tiles per chunk / 8
LOG_2PI = float(np.log(2.0 * np.pi))

_CACHE = {}


def _chunks():
    out = []
    c0 = 0
    while c0 < N:
        csz = min(CH, N - c0)
        out.append((c0, csz))
        c0 += csz
    return out


def _build_nc(reps=1, loop_iters=None, skip_act=False, skip_mm=False,
              skip_tp=False, max_chunks=None):
    from concourse import bacc, mybir, masks, tile

    f32 = mybir.dt.float32
    f16 = mybir.dt.float16
    nc = bacc.Bacc("TRN2", target_bir_lowering=False, debug=False,
                   num_devices=NCORES)

    x_eval = nc.dram_tensor("x_eval", [MS, D], f32, kind="ExternalInput")
    x_base = nc.dram_tensor("x_base", [N, D], f32, kind="ExternalInput")
    sc_in = nc.dram_tensor("sc", [1, 2], f32, kind="ExternalInput")
    out = nc.dram_tensor("out", [128, RT], f32, kind="ExternalOutput")

    chunks = _chunks()
    NCH = len(chunks)
    Exp = mybir.ActivationFunctionType.Exp
    ADD = mybir.AluOpType.add
    MULT = mybir.AluOpType.mult
    X = mybir.AxisListType.X

    with tile.TileContext(nc) as tc:
        with (
            tc.tile_pool(name="persist", bufs=1) as pp,
            tc.tile_pool(name="bs", bufs=2) as bsp,
            tc.tile_pool(name="rhs", bufs=2) as rhsp,
            tc.tile_pool(name="mm", bufs=2, space="PSUM") as mmp,
            tc.tile_pool(name="tp", bufs=2, space="PSUM") as tpp,
        ):
          from contextlib import nullcontext
          for _rep in range(reps):
           with (tc.For_i(0, loop_iters, 1) if loop_iters else nullcontext()):
            identity = pp.tile([128, 128], f16)
            masks.make_identity(nc, identity[:])

            # ---- host-precomputed scalars [scale, c0 - log_bw] ----------
            sc_sb = pp.tile([1, 2], f32)
            nc.sync.dma_start(out=sc_sb[:], in_=sc_in[:])
            scale_col = pp.tile([128, 1], f32)
            nc.gpsimd.partition_broadcast(scale_col[:], sc_sb[:, 0:1])
            c_col = pp.tile([128, 1], f32)
            nc.gpsimd.partition_broadcast(c_col[:], sc_sb[:, 1:2])

            # ---- eval-side setup ----------------------------------------
            ev_nat = pp.tile([128, RT * D], f32)
            nc.sync.dma_start(
                out=ev_nat[:].rearrange("p (t d) -> p t d", d=D),
                in_=x_eval[:].rearrange("(p t) d -> p t d", p=128))
            ev_sq = pp.tile([128, RT * D], f32)
            nc.vector.tensor_mul(ev_sq[:], ev_nat[:], ev_nat[:])
            sq_e = pp.tile([128, RT], f32)
            nc.vector.tensor_reduce(
                out=sq_e[:], in_=ev_sq[:].rearrange("p (t d) -> p t d", d=D),
                axis=X, op=ADD)
            # bias_all[:, rt] = scale*|e|^2 + (c0 - log_bw)
            bias_all = pp.tile([128, RT], f32)
            nc.vector.tensor_scalar(out=bias_all[:], in0=sq_e[:],
                                    scalar1=scale_col[:, 0:1],
                                    scalar2=c_col[:, 0:1],
                                    op0=MULT, op1=ADD)

            # hi/lo split of -2*eval, packed for 32-row stacked transposes:
            #   ee_nat[p, rt, 0:16]=he, [16:32]=he   -> evT16 rows 0:32
            #   lo_nat[p, rt, 0:16]=le, [16:32]=1.0  -> evT16 rows 32:64
            e2f = pp.tile([128, RT * D], f32)
            nc.vector.tensor_scalar_mul(e2f[:], ev_nat[:], -2.0)
            ee_nat = pp.tile([128, RT * 2 * D], f16)
            ee_r = ee_nat[:].rearrange("p (t d) -> p t d", d=2 * D)
            e2f_r = e2f[:].rearrange("p (t d) -> p t d", d=D)
            nc.vector.tensor_copy(ee_r[:, :, 0:D], e2f_r)
            nc.vector.tensor_copy(ee_r[:, :, D:2 * D], ee_r[:, :, 0:D])
            e2hf = pp.tile([128, RT * D], f32)
            nc.vector.tensor_copy(e2hf[:], ee_r[:, :, 0:D])
            lo_nat = pp.tile([128, RT * 2 * D], f16)
            nc.vector.memset(lo_nat[:], 1.0)
            lo_r = lo_nat[:].rearrange("p (t d) -> p t d", d=2 * D)
            nc.vector.tensor_sub(lo_r[:, :, 0:D], e2f[:].rearrange(
                "p (t d) -> p t d", d=D), e2hf[:].rearrange(
                "p (t d) -> p t d", d=D))

            # evT16 rows: 0:16 he, 16:32 he, 32:48 le, 48:80 ones
            evT16 = pp.tile([80, MS], f16)
            nc.vector.memset(evT16[:], 1.0)
            tpe_h = tpp.tile([32, 1024], f16, tag="tp")
            tpe_l = tpp.tile([32, 1024], f16, tag="tp")
            for rt in range(RT):
                if not skip_tp:
                    nc.tensor.transpose(tpe_h[:, rt * 128:(rt + 1) * 128],
                                        ee_nat[:, rt * 32:(rt + 1) * 32],
                                        identity[:])
                    nc.tensor.transpose(tpe_l[:, rt * 128:(rt + 1) * 128],
                                        lo_nat[:, rt * 32:(rt + 1) * 32],
                                        identity[:])
            nc.vector.tensor_copy(evT16[0:32, :], tpe_h[:])
            nc.vector.tensor_copy(evT16[32:64, :], tpe_l[:])

            # ---- main loop: stream base columns per chunk ---------------
            sums = pp.tile([128, RT * NCH], f32)
            if skip_act or (max_chunks is not None and max_chunks < NCH):
                nc.vector.memset(sums[:], 0.0)
            xb_r = x_base[:].rearrange("(p t) d -> p t d", p=128)
            for ci, (cs, csz) in enumerate(chunks[:max_chunks]):
                nt = csz // 128
                t0 = cs // 128
                bs_c = bsp.tile([128, 12 * D], f32, tag="bs")
                nc.sync.dma_start(
                    out=bs_c[:, 0:nt * D].rearrange("p (t d) -> p t d", d=D),
                    in_=xb_r[:, t0:t0 + nt, :])
                w = nt * D
                bs_r = bs_c[:, 0:w].rearrange("p (t d) -> p t d", d=D)
                # hl[p, t, 0:16]=hb, [16:32]=lb  -> rhs rows 0:32
                # hp[p, t, 0:16]=hb, [16:32]=ph  -> rhs rows 32:64
                # pl[p, t, 0:16]=pl              -> rhs rows 64:80
                hl = bsp.tile([128, 12 * 2 * D], f16, tag="hl")
                hl_r = hl[:, 0:2 * w].rearrange("p (t d) -> p t d", d=2 * D)
                nc.vector.tensor_copy(hl_r[:, :, 0:D], bs_r)
                hbf = bsp.tile([128, 12 * D], f32, tag="hbf")
                hbf_r = hbf[:, 0:w].rearrange("p (t d) -> p t d", d=D)
                nc.vector.tensor_copy(hbf_r, hl_r[:, :, 0:D])
                nc.vector.tensor_sub(hl_r[:, :, D:2 * D], bs_r, hbf_r)
                hp = bsp.tile([128, 12 * 2 * D], f16, tag="hp")
                hp_r = hp[:, 0:2 * w].rearrange("p (t d) -> p t d", d=2 * D)
                nc.vector.tensor_copy(hp_r[:, :, 0:D], hl_r[:, :, 0:D])
                b2 = bsp.tile([128, 12 * D], f32, tag="b2")
                nc.vector.tensor_mul(b2[:, 0:w], bs_c[:, 0:w], bs_c[:, 0:w])
                b2_r = b2[:, 0:w].rearrange("p (t d) -> p t d", d=D)
                nc.vector.tensor_copy(hp_r[:, :, D:2 * D], b2_r)
                phf = bsp.tile([128, 12 * D], f32, tag="phf")
                phf_r = phf[:, 0:w].rearrange("p (t d) -> p t d", d=D)
                nc.vector.tensor_copy(phf_r, hp_r[:, :, D:2 * D])
                pl = bsp.tile([128, 12 * D], f16, tag="pl")
                nc.vector.tensor_sub(pl[:, 0:w], b2[:, 0:w], phf[:, 0:w])

                rhs = rhsp.tile([80, CH], f16, tag="rhs")
                for src, dst, sw in ((hl, 0, 2 * D), (hp, 32, 2 * D),
                                     (pl, 64, D)):
                    np_t = 16 * (sw // D)  # out partitions per transpose
                    for b0 in range(0, nt, 8):
                        bw_t = min(8, nt - b0)
                        tp = tpp.tile([32, 1024], f16, tag="tp")
                        for j in range(bw_t):
                            if not skip_tp:
                                nc.tensor.transpose(
                                    tp[0:np_t, j * 128:(j + 1) * 128],
                                    src[:, (b0 + j) * sw:(b0 + j + 1) * sw],
                                    identity[:])
                        nc.vector.tensor_copy(
                            rhs[dst:dst + np_t,
                                b0 * 128:(b0 + bw_t) * 128],
                            tp[0:np_t, 0:bw_t * 128])
                for rt in range(RT):
                    ps = mmp.tile([128, CH], f32, tag="mm")
                    if not skip_mm:
                        for j in range(csz // 512):
                            nc.tensor.matmul(
                                ps[:, j * 512:(j + 1) * 512],
                                evT16[0:80, rt * 128:(rt + 1) * 128],
                                rhs[0:80, j * 512:(j + 1) * 512],
                                start=True, stop=True)
                    if not skip_act:
                        nc.scalar.activation(
                            ps[:, 0:csz], ps[:, 0:csz], Exp,
                            bias=bias_all[:, rt:rt + 1],
                            scale=scale_col[:, 0:1],
                            accum_out=sums[:, rt * NCH + ci:rt * NCH + ci + 1])

            # ---- finalize -----------------------------------------------
            val = pp.tile([128, RT], f32)
            for rt in range(RT):
                nc.vector.tensor_reduce(
                    out=val[:, rt:rt + 1],
                    in_=sums[:, rt * NCH:(rt + 1) * NCH], axis=X, op=ADD)
            nc.sync.dma_start(out=out[:], in_=val[:])

    nc.compile()
    return nc


def _in_maps(x_eval, x_base, log_bw):
    x_eval = np.ascontiguousarray(x_eval, dtype=np.float32)
    x_base = np.ascontiguousarray(x_base, dtype=np.float32)
    lbv = float(np.asarray(log_bw).reshape(-1)[0])
    scale = -0.5 * float(np.exp(-2.0 * lbv))
    c = -0.5 * D * LOG_2PI - float(np.log(N)) - lbv
    sc = np.array([[scale, c]], dtype=np.float32)
    return [
        {
            "x_eval": x_eval[i * MS:(i + 1) * MS],
            "x_base": x_base,
            "sc": sc,
        }
        for i in range(NCORES)
    ]


def kernel(x_eval, x_base, log_bw):
    from concourse.bass_utils import run_bass_kernel_spmd

    if "nc" not in _CACHE:
        _CACHE["nc"] = _build_nc()
    nc = _CACHE["nc"]

    in_maps = _in_maps(x_eval, x_base, log_bw)
    res = run_bass_kernel_spmd(nc, in_maps, list(range(NCORES)))
    # out[p, rt] holds eval point p*RT + rt of the shard -> row-major flatten
    shards = [r["out"].reshape(-1) for r in res.results]
    return np.concatenate(shards).astype(np.float32)


# revision 11
# speedup vs baseline: 1.2177x; 1.1136x over previous
"""Trainium2 Bass kernel for Gaussian KDE evaluation.

reference math:
    val[m] = (1/N) * sum_n exp(t1 - 0.5*d2(m,n)/bw^2)
    d2(m,n) = |e_m|^2 + |b_n|^2 - 2<e_m, b_n>
    t1 = -0.5*D*log(2*pi) - log_bw,  bw^2 = exp(2*log_bw)

Strategy (8 NeuronCores, x_eval row-sharded, x_base/log_bw replicated):
  All matmuls run in fp16 (1 PE cycle/row vs 4 for fp32) using an exact
  hi/lo error split so d2 keeps ~2^-22 effective precision:
    cross  = he*hb + le*hb + he*lb   (he=f16(-2e), le=f16(-2e-he), ...)
    |b|^2  = ones*P_hi + ones*P_lo   (P_hi=f16(b^2), P_lo=f16(b^2-P_hi))
  One K=80 matmul per 512-col tile produces |b|^2 - 2<e,b> in PSUM; a
  single ScalarE ACTIVATE computes exp(scale*psum + bias) in place with
  scale = -0.5/bw^2, per-partition bias = t1 - ln(N) + scale*|e_m|^2,
  and its accum_out emits the row-sum.  Base columns stream per chunk:
  DMA slice -> DVE casts -> PE transposes -> rhs tile, fully overlapped
  with the ACT-bound main loop.  The log_bw scalar chain (scale and bias
  constant) is precomputed on host in _in_maps and broadcast on-device
  via gpsimd.partition_broadcast, keeping PSUM free for the pipeline.
"""

import numpy as np

M, N, D = 8192, 16384, 16
NCORES = 8
MS = M // NCORES          # eval rows per core
RT = MS // 128            # row tiles per core (128 evals each)
CH = 1536                 # column-chunk size (3 PSUM banks)
TPC = CH // D // 8        # nominal tiles per chunk / 8
LOG_2PI = float(np.log(2.0 * np.pi))

_CACHE = {}


def _chunks():
    out = []
    c0 = 0
    while c0 < N:
        csz = min(CH, N - c0)
        out.append((c0, csz))
        c0 += csz
    return out


def _build_nc(reps=1, loop_iters=None, skip_act=False, skip_mm=False,
              skip_tp=False, max_chunks=None):
    from concourse import bacc, mybir, masks, tile

    f32 = mybir.dt.float32
    f16 = mybir.dt.float16
    nc = bacc.Bacc("TRN2", target_bir_lowering=False, debug=False,
                   num_devices=NCORES)

    x_eval = nc.dram_tensor("x_eval", [MS, D], f32, kind="ExternalInput")
    x_base = nc.dram_tensor("x_base", [N, D], f32, kind="ExternalInput")
    sc_in = nc.dram_tensor("sc", [1, 2], f32, kind="ExternalInput")
    out = nc.dram_tensor("out", [128, RT], f32, kind="ExternalOutput")

    chunks = _chunks()
    NCH = len(chunks)
    Exp = mybir.ActivationFunctionType.Exp
    ADD = mybir.AluOpType.add
    MULT = mybir.AluOpType.mult
    X = mybir.AxisListType.X

    with tile.TileContext(nc) as tc:
        with (
            tc.tile_pool(name="persist", bufs=1) as pp,
            tc.tile_pool(name="bs", bufs=3) as bsp,
            tc.tile_pool(name="rhs", bufs=3) as rhsp,
            tc.tile_pool(name="mm", bufs=2, space="PSUM") as mmp,
            tc.tile_pool(name="tp", bufs=2, space="PSUM") as tpp,
        ):
          from contextlib import nullcontext
          for _rep in range(reps):
           with (tc.For_i(0, loop_iters, 1) if loop_iters else nullcontext()):
            identity = pp.tile([128, 128], f16)
            masks.make_identity(nc, identity[:])

            # ---- host-precomputed scalars [scale, c0 - log_bw] ----------
            sc_sb = pp.tile([1, 2], f32)
            nc.sync.dma_start(out=sc_sb[:], in_=sc_in[:])
            scale_col = pp.tile([128, 1], f32)
            nc.gpsimd.partition_broadcast(scale_col[:], sc_sb[:, 0:1])
            c_col = pp.tile([128, 1], f32)
            nc.gpsimd.partition_broadcast(c_col[:], sc_sb[:, 1:2])

            # ---- eval-side setup ----------------------------------------
            ev_nat = pp.tile([128, RT * D], f32)
            nc.sync.dma_start(
                out=ev_nat[:].rearrange("p (t d) -> p t d", d=D),
                in_=x_eval[:].rearrange("(p t) d -> p t d", p=128))
            ev_sq = pp.tile([128, RT * D], f32)
            nc.vector.tensor_mul(ev_sq[:], ev_nat[:], ev_nat[:])
            sq_e = pp.tile([128, RT], f32)
            nc.vector.tensor_reduce(
                out=sq_e[:], in_=ev_sq[:].rearrange("p (t d) -> p t d", d=D),
                axis=X, op=ADD)
            # bias_all[:, rt] = scale*|e|^2 + (c0 - log_bw)
            bias_all = pp.tile([128, RT], f32)
            nc.vector.tensor_scalar(out=bias_all[:], in0=sq_e[:],
                                    scalar1=scale_col[:, 0:1],
                                    scalar2=c_col[:, 0:1],
                                    op0=MULT, op1=ADD)

            # hi/lo split of -2*eval, packed for 32-row stacked transposes:
            #   ee_nat[p, rt, 0:16]=he, [16:32]=he   -> evT16 rows 0:32
            #   lo_nat[p, rt, 0:16]=le, [16:32]=1.0  -> evT16 rows 32:64
            e2f = pp.tile([128, RT * D], f32)
            nc.vector.tensor_scalar_mul(e2f[:], ev_nat[:], -2.0)
            ee_nat = pp.tile([128, RT * 2 * D], f16)
            ee_r = ee_nat[:].rearrange("p (t d) -> p t d", d=2 * D)
            e2f_r = e2f[:].rearrange("p (t d) -> p t d", d=D)
            nc.vector.tensor_copy(ee_r[:, :, 0:D], e2f_r)
            nc.vector.tensor_copy(ee_r[:, :, D:2 * D], ee_r[:, :, 0:D])
            e2hf = pp.tile([128, RT * D], f32)
            nc.vector.tensor_copy(e2hf[:], ee_r[:, :, 0:D])
            lo_nat = pp.tile([128, RT * 2 * D], f16)
            nc.vector.memset(lo_nat[:], 1.0)
            lo_r = lo_nat[:].rearrange("p (t d) -> p t d", d=2 * D)
            nc.vector.tensor_sub(lo_r[:, :, 0:D], e2f[:].rearrange(
                "p (t d) -> p t d", d=D), e2hf[:].rearrange(
                "p (t d) -> p t d", d=D))

            # evT16 rows: 0:16 he, 16:32 he, 32:48 le, 48:80 ones
            evT16 = pp.tile([80, MS], f16)
            nc.vector.memset(evT16[:], 1.0)
            tpe_h = tpp.tile([32, 1024], f16, tag="tp")
            tpe_l = tpp.tile([32, 1024], f16, tag="tp")
            for rt in range(RT):
                if not skip_tp:
                    nc.tensor.transpose(tpe_h[:, rt * 128:(rt + 1) * 128],
                                        ee_nat[:, rt * 32:(rt + 1) * 32],
                                        identity[:])
                    nc.tensor.transpose(tpe_l[:, rt * 128:(rt + 1) * 128],
                                        lo_nat[:, rt * 32:(rt + 1) * 32],
                                        identity[:])
            nc.vector.tensor_copy(evT16[0:32, :], tpe_h[:])
            nc.vector.tensor_copy(evT16[32:64, :], tpe_l[:])

            # ---- main loop: stream base columns per chunk ---------------
            sums = pp.tile([128, RT * NCH], f32)
            if skip_act or (max_chunks is not None and max_chunks < NCH):
                nc.vector.memset(sums[:], 0.0)
            xb_r = x_base[:].rearrange("(p t) d -> p t d", p=128)
            for ci, (cs, csz) in enumerate(chunks[:max_chunks]):
                nt = csz // 128
                t0 = cs // 128
                bs_c = bsp.tile([128, 12 * D], f32, tag="bs")
                nc.sync.dma_start(
                    out=bs_c[:, 0:nt * D].rearrange("p (t d) -> p t d", d=D),
                    in_=xb_r[:, t0:t0 + nt, :])
                w = nt * D
                bs_r = bs_c[:, 0:w].rearrange("p (t d) -> p t d", d=D)
                # hl[p, t, 0:16]=hb, [16:32]=lb  -> rhs rows 0:32
                # hp[p, t, 0:16]=hb, [16:32]=ph  -> rhs rows 32:64
                # pl[p, t, 0:16]=pl              -> rhs rows 64:80
                hl = bsp.tile([128, 12 * 2 * D], f16, tag="hl")
                hl_r = hl[:, 0:2 * w].rearrange("p (t d) -> p t d", d=2 * D)
                nc.vector.tensor_copy(hl_r[:, :, 0:D], bs_r)
                hbf = bsp.tile([128, 12 * D], f32, tag="hbf")
                hbf_r = hbf[:, 0:w].rearrange("p (t d) -> p t d", d=D)
                nc.vector.tensor_copy(hbf_r, hl_r[:, :, 0:D])
                nc.vector.tensor_sub(hl_r[:, :, D:2 * D], bs_r, hbf_r)
                hp = bsp.tile([128, 12 * 2 * D], f16, tag="hp")
                hp_r = hp[:, 0:2 * w].rearrange("p (t d) -> p t d", d=2 * D)
                nc.vector.tensor_copy(hp_r[:, :, 0:D], hl_r[:, :, 0:D])
                b2 = bsp.tile([128, 12 * D], f32, tag="b2")
                nc.vector.tensor_mul(b2[:, 0:w], bs_c[:, 0:w], bs_c[:, 0:w])
                b2_r = b2[:, 0:w].rearrange("p (t d) -> p t d", d=D)
                nc.vector.tensor_copy(hp_r[:, :, D:2 * D], b2_r)
                phf = bsp.tile([128, 12 * D], f32, tag="phf")
                phf_r = phf[:, 0:w].rearrange("p (t d) -> p t d", d=D)
                nc.vector.tensor_copy(phf_r, hp_r[:, :, D:2 * D])
                pl = bsp.tile([128, 12 * D], f16, tag="pl")
                nc.vector.tensor_sub(pl[:, 0:w], b2[:, 0:w], phf[:, 0:w])

                rhs = rhsp.tile([80, CH], f16, tag="rhs")
                for src, dst, sw in ((hl, 0, 2 * D), (hp, 32, 2 * D),
                                     (pl, 64, D)):
                    np_t = 16 * (sw // D)  # out partitions per transpose
                    for b0 in range(0, nt, 8):
                        bw_t = min(8, nt - b0)
                        tp = tpp.tile([32, 1024], f16, tag="tp")
                        for j in range(bw_t):
                            if not skip_tp:
                                nc.tensor.transpose(
                                    tp[0:np_t, j * 128:(j + 1) * 128],
                                    src[:, (b0 + j) * sw:(b0 + j + 1) * sw],
                                    identity[:])
                        nc.vector.tensor_copy(
                            rhs[dst:dst + np_t,
                                b0 * 128:(b0 + bw_t) * 128],
                            tp[0:np_t, 0:bw_t * 128])
                for rt in range(RT):
                    ps = mmp.tile([128, CH], f32, tag="mm")
                    if not skip_mm:
                        for j in range(csz // 512):
                            nc.tensor.matmul(
                                ps[:, j * 512:(j + 1) * 512],
                                evT16[0:80, rt * 128:(rt + 1) * 128],
                                rhs[0:80, j * 512:(j + 1) * 512],
                                start=True, stop=True)
                    if not skip_act:
                        nc.scalar.activation(
                            ps[:, 0:csz], ps[:, 0:csz], Exp,
                            bias=bias_all[:, rt:rt + 1],
                            scale=scale_col[:, 0:1],
                            accum_out=sums[:, rt * NCH + ci:rt * NCH + ci + 1])

            # ---- finalize -----------------------------------------------
            val = pp.tile([128, RT], f32)
            for rt in range(RT):
                nc.vector.tensor_reduce(
                    out=val[:, rt:rt + 1],
                    in_=sums[:, rt * NCH:(rt + 1) * NCH], axis=X, op=ADD)
            nc.sync.dma_start(out=out[:], in_=val[:])

    nc.compile()
    return nc


def _in_maps(x_eval, x_base, log_bw):
    x_eval = np.ascontiguousarray(x_eval, dtype=np.float32)
    x_base = np.ascontiguousarray(x_base, dtype=np.float32)
    lbv = float(np.asarray(log_bw).reshape(-1)[0])
    scale = -0.5 * float(np.exp(-2.0 * lbv))
    c = -0.5 * D * LOG_2PI - float(np.log(N)) - lbv
    sc = np.array([[scale, c]], dtype=np.float32)
    return [
        {
            "x_eval": x_eval[i * MS:(i + 1) * MS],
            "x_base": x_base,
            "sc": sc,
        }
        for i in range(NCORES)
    ]


def kernel(x_eval, x_base, log_bw):
    from concourse.bass_utils import run_bass_kernel_spmd

    if "nc" not in _CACHE:
        _CACHE["nc"] = _build_nc()
    nc = _CACHE["nc"]

    in_maps = _in_maps(x_eval, x_base, log_bw)
    res = run_bass_kernel_spmd(nc, in_maps, list(range(NCORES)))
    # out[p, rt] holds eval point p*RT + rt of the shard -> row-major flatten
    shards = [r["out"].reshape(-1) for r in res.results]
    return np.concatenate(shards).astype(np.float32)


# revision 12
# speedup vs baseline: 5.7344x; 4.7092x over previous
"""Trainium2 Bass kernel for Gaussian KDE evaluation.

reference math:
    val[m] = (1/N) * sum_n exp(t1 - 0.5*d2(m,n)/bw^2)
    d2(m,n) = |e_m|^2 + |b_n|^2 - 2<e_m, b_n>
    t1 = -0.5*D*log(2*pi) - log_bw,  bw^2 = exp(2*log_bw)

Strategy (8 NeuronCores, x_eval row-sharded, x_base/log_bw replicated):
  All matmuls run in fp16 (1 PE cycle/row vs 4 for fp32) using an exact
  hi/lo error split so d2 keeps ~2^-22 effective precision:
    cross  = he*hb + le*hb + he*lb   (he=f16(-2e), le=f16(-2e-he), ...)
    |b|^2  = ones*P_hi + ones*P_lo   (P_hi=f16(b^2), P_lo=f16(b^2-P_hi))
  One K=80 matmul per 512-col tile produces |b|^2 - 2<e,b> in PSUM; a
  single ScalarE ACTIVATE computes exp(scale*psum + bias) in place with
  scale = -0.5/bw^2, per-partition bias = t1 - ln(N) + scale*|e_m|^2,
  and its accum_out emits the row-sum.  Base columns stream per chunk:
  DMA slice -> DVE casts -> PE transposes -> rhs tile, fully overlapped
  with the ACT-bound main loop.  The log_bw scalar chain (scale and bias
  constant) is precomputed on host in _in_maps and broadcast on-device
  via gpsimd.partition_broadcast, keeping PSUM free for the pipeline.
"""

import numpy as np

M, N, D = 8192, 16384, 16
NCORES = 8
MS = M // NCORES          # eval rows per core
RT = MS // 128            # row tiles per core (128 evals each)
CH = 1536                 # column-chunk size (3 PSUM banks)
TPC = CH // D // 8        # nominal tiles per chunk / 8
LOG_2PI = float(np.log(2.0 * np.pi))

_CACHE = {}


def _chunks():
    out = []
    c0 = 0
    while c0 < N:
        csz = min(CH, N - c0)
        out.append((c0, csz))
        c0 += csz
    return out


def _build_nc(reps=1, loop_iters=None, skip_act=False, skip_mm=False,
              skip_tp=False, max_chunks=None):
    from concourse import bacc, mybir, masks, tile

    f32 = mybir.dt.float32
    f16 = mybir.dt.float16
    nc = bacc.Bacc("TRN2", target_bir_lowering=False, debug=False,
                   num_devices=NCORES)

    x_eval = nc.dram_tensor("x_eval", [MS, D], f32, kind="ExternalInput")
    x_base = nc.dram_tensor("x_base", [N, D], f32, kind="ExternalInput")
    sc_in = nc.dram_tensor("sc", [1, 2], f32, kind="ExternalInput")
    out = nc.dram_tensor("out", [128, RT], f32, kind="ExternalOutput")

    chunks = _chunks()
    NCH = len(chunks)
    Exp = mybir.ActivationFunctionType.Exp
    ADD = mybir.AluOpType.add
    MULT = mybir.AluOpType.mult
    X = mybir.AxisListType.X

    with tile.TileContext(nc) as tc:
        with (
            tc.tile_pool(name="persist", bufs=1) as pp,
            tc.tile_pool(name="bs", bufs=4) as bsp,
            tc.tile_pool(name="rhs", bufs=4) as rhsp,
            tc.tile_pool(name="mm", bufs=2, space="PSUM") as mmp,
            tc.tile_pool(name="tp", bufs=2, space="PSUM") as tpp,
        ):
          from contextlib import nullcontext
          for _rep in range(reps):
           with (tc.For_i(0, loop_iters, 1) if loop_iters else nullcontext()):
            identity = pp.tile([128, 128], f16)
            masks.make_identity(nc, identity[:])

            # ---- host-precomputed scalars [scale, c0 - log_bw] ----------
            sc_sb = pp.tile([1, 2], f32)
            nc.sync.dma_start(out=sc_sb[:], in_=sc_in[:])
            scale_col = pp.tile([128, 1], f32)
            nc.gpsimd.partition_broadcast(scale_col[:], sc_sb[:, 0:1])
            c_col = pp.tile([128, 1], f32)
            nc.gpsimd.partition_broadcast(c_col[:], sc_sb[:, 1:2])

            # ---- eval-side setup ----------------------------------------
            ev_nat = pp.tile([128, RT * D], f32)
            nc.sync.dma_start(
                out=ev_nat[:].rearrange("p (t d) -> p t d", d=D),
                in_=x_eval[:].rearrange("(p t) d -> p t d", p=128))
            ev_sq = pp.tile([128, RT * D], f32)
            nc.vector.tensor_mul(ev_sq[:], ev_nat[:], ev_nat[:])
            sq_e = pp.tile([128, RT], f32)
            nc.vector.tensor_reduce(
                out=sq_e[:], in_=ev_sq[:].rearrange("p (t d) -> p t d", d=D),
                axis=X, op=ADD)
            # bias_all[:, rt] = scale*|e|^2 + (c0 - log_bw)
            bias_all = pp.tile([128, RT], f32)
            nc.vector.tensor_scalar(out=bias_all[:], in0=sq_e[:],
                                    scalar1=scale_col[:, 0:1],
                                    scalar2=c_col[:, 0:1],
                                    op0=MULT, op1=ADD)

            # hi/lo split of -2*eval, packed for 32-row stacked transposes:
            #   ee_nat[p, rt, 0:16]=he, [16:32]=he   -> evT16 rows 0:32
            #   lo_nat[p, rt, 0:16]=le, [16:32]=1.0  -> evT16 rows 32:64
            e2f = pp.tile([128, RT * D], f32)
            nc.vector.tensor_scalar_mul(e2f[:], ev_nat[:], -2.0)
            ee_nat = pp.tile([128, RT * 2 * D], f16)
            ee_r = ee_nat[:].rearrange("p (t d) -> p t d", d=2 * D)
            e2f_r = e2f[:].rearrange("p (t d) -> p t d", d=D)
            nc.vector.tensor_copy(ee_r[:, :, 0:D], e2f_r)
            nc.vector.tensor_copy(ee_r[:, :, D:2 * D], ee_r[:, :, 0:D])
            e2hf = pp.tile([128, RT * D], f32)
            nc.vector.tensor_copy(e2hf[:], ee_r[:, :, 0:D])
            lo_nat = pp.tile([128, RT * 2 * D], f16)
            nc.vector.memset(lo_nat[:], 1.0)
            lo_r = lo_nat[:].rearrange("p (t d) -> p t d", d=2 * D)
            nc.vector.tensor_sub(lo_r[:, :, 0:D], e2f[:].rearrange(
                "p (t d) -> p t d", d=D), e2hf[:].rearrange(
                "p (t d) -> p t d", d=D))

            # evT16 rows: 0:16 he, 16:32 he, 32:48 le, 48:80 ones
            evT16 = pp.tile([80, MS], f16)
            nc.vector.memset(evT16[:], 1.0)
            tpe_h = tpp.tile([32, 1024], f16, tag="tp")
            tpe_l = tpp.tile([32, 1024], f16, tag="tp")
            for rt in range(RT):
                if not skip_tp:
                    nc.tensor.transpose(tpe_h[:, rt * 128:(rt + 1) * 128],
                                        ee_nat[:, rt * 32:(rt + 1) * 32],
                                        identity[:])
                    nc.tensor.transpose(tpe_l[:, rt * 128:(rt + 1) * 128],
                                        lo_nat[:, rt * 32:(rt + 1) * 32],
                                        identity[:])
            nc.vector.tensor_copy(evT16[0:32, :], tpe_h[:])
            nc.vector.tensor_copy(evT16[32:64, :], tpe_l[:])

            # ---- main loop: stream base columns per chunk ---------------
            sums = pp.tile([128, RT * NCH], f32)
            if skip_act or (max_chunks is not None and max_chunks < NCH):
                nc.vector.memset(sums[:], 0.0)
            xb_r = x_base[:].rearrange("(p t) d -> p t d", p=128)
            for ci, (cs, csz) in enumerate(chunks[:max_chunks]):
                nt = csz // 128
                t0 = cs // 128
                bs_c = bsp.tile([128, 12 * D], f32, tag="bs")
                nc.sync.dma_start(
                    out=bs_c[:, 0:nt * D].rearrange("p (t d) -> p t d", d=D),
                    in_=xb_r[:, t0:t0 + nt, :])
                w = nt * D
                bs_r = bs_c[:, 0:w].rearrange("p (t d) -> p t d", d=D)
                # hl[p, t, 0:16]=hb, [16:32]=lb  -> rhs rows 0:32
                # hp[p, t, 0:16]=hb, [16:32]=ph  -> rhs rows 32:64
                # pl[p, t, 0:16]=pl              -> rhs rows 64:80
                hl = bsp.tile([128, 12 * 2 * D], f16, tag="hl")
                hl_r = hl[:, 0:2 * w].rearrange("p (t d) -> p t d", d=2 * D)
                nc.vector.tensor_copy(hl_r[:, :, 0:D], bs_r)
                hbf = bsp.tile([128, 12 * D], f32, tag="hbf")
                hbf_r = hbf[:, 0:w].rearrange("p (t d) -> p t d", d=D)
                nc.vector.tensor_copy(hbf_r, hl_r[:, :, 0:D])
                nc.vector.tensor_sub(hl_r[:, :, D:2 * D], bs_r, hbf_r)
                hp = bsp.tile([128, 12 * 2 * D], f16, tag="hp")
                hp_r = hp[:, 0:2 * w].rearrange("p (t d) -> p t d", d=2 * D)
                nc.vector.tensor_copy(hp_r[:, :, 0:D], hl_r[:, :, 0:D])
                b2 = bsp.tile([128, 12 * D], f32, tag="b2")
                nc.vector.tensor_mul(b2[:, 0:w], bs_c[:, 0:w], bs_c[:, 0:w])
                b2_r = b2[:, 0:w].rearrange("p (t d) -> p t d", d=D)
                nc.vector.tensor_copy(hp_r[:, :, D:2 * D], b2_r)
                phf = bsp.tile([128, 12 * D], f32, tag="phf")
                phf_r = phf[:, 0:w].rearrange("p (t d) -> p t d", d=D)
                nc.vector.tensor_copy(phf_r, hp_r[:, :, D:2 * D])
                pl = bsp.tile([128, 12 * D], f16, tag="pl")
                nc.vector.tensor_sub(pl[:, 0:w], b2[:, 0:w], phf[:, 0:w])

                rhs = rhsp.tile([80, CH], f16, tag="rhs")
                for src, dst, sw in ((hl, 0, 2 * D), (hp, 32, 2 * D),
                                     (pl, 64, D)):
                    np_t = 16 * (sw // D)  # out partitions per transpose
                    for b0 in range(0, nt, 8):
                        bw_t = min(8, nt - b0)
                        tp = tpp.tile([32, 1024], f16, tag="tp")
                        for j in range(bw_t):
                            if not skip_tp:
                                nc.tensor.transpose(
                                    tp[0:np_t, j * 128:(j + 1) * 128],
                                    src[:, (b0 + j) * sw:(b0 + j + 1) * sw],
                                    identity[:])
                        nc.vector.tensor_copy(
                            rhs[dst:dst + np_t,
                                b0 * 128:(b0 + bw_t) * 128],
                            tp[0:np_t, 0:bw_t * 128])
                for rt in range(RT):
                    ps = mmp.tile([128, CH], f32, tag="mm")
                    if not skip_mm:
                        for j in range(csz // 512):
                            nc.tensor.matmul(
                                ps[:, j * 512:(j + 1) * 512],
                                evT16[0:80, rt * 128:(rt + 1) * 128],
                                rhs[0:80, j * 512:(j + 1) * 512],
                                start=True, stop=True)
                    if not skip_act:
                        nc.scalar.activation(
                            ps[:, 0:csz], ps[:, 0:csz], Exp,
                            bias=bias_all[:, rt:rt + 1],
                            scale=scale_col[:, 0:1],
                            accum_out=sums[:, rt * NCH + ci:rt * NCH + ci + 1])

            # ---- finalize -----------------------------------------------
            val = pp.tile([128, RT], f32)
            for rt in range(RT):
                nc.vector.tensor_reduce(
                    out=val[:, rt:rt + 1],
                    in_=sums[:, rt * NCH:(rt + 1) * NCH], axis=X, op=ADD)
            nc.sync.dma_start(out=out[:], in_=val[:])

    nc.compile()
    return nc


def _in_maps(x_eval, x_base, log_bw):
    x_eval = np.ascontiguousarray(x_eval, dtype=np.float32)
    x_base = np.ascontiguousarray(x_base, dtype=np.float32)
    lbv = float(np.asarray(log_bw).reshape(-1)[0])
    scale = -0.5 * float(np.exp(-2.0 * lbv))
    c = -0.5 * D * LOG_2PI - float(np.log(N)) - lbv
    sc = np.array([[scale, c]], dtype=np.float32)
    return [
        {
            "x_eval": x_eval[i * MS:(i + 1) * MS],
            "x_base": x_base,
            "sc": sc,
        }
        for i in range(NCORES)
    ]


def kernel(x_eval, x_base, log_bw):
    from concourse.bass_utils import run_bass_kernel_spmd

    if "nc" not in _CACHE:
        _CACHE["nc"] = _build_nc()
    nc = _CACHE["nc"]

    in_maps = _in_maps(x_eval, x_base, log_bw)
    res = run_bass_kernel_spmd(nc, in_maps, list(range(NCORES)))
    # out[p, rt] holds eval point p*RT + rt of the shard -> row-major flatten
    shards = [r["out"].reshape(-1) for r in res.results]
    return np.concatenate(shards).astype(np.float32)
